# revision 2
# baseline (speedup 1.0000x reference)
"""Trainium2 Bass kernel for nn_ANN_Comp_29240137351521 (dense_cnn).

Reference computes, per batch row b of x [16384, 512] (complex, given as
real/imag f32 pairs):
    h = x @ w0                      # [B, 512] complex
    a = ifft(fft(h, n=1023)^2)      # full self-convolution, [B, 1023]
    out = |a @ wlast|               # [B, 10] f32

Algebraic collapse used here: the self-convolution + final contraction is a
polynomial-evaluation identity. With L = 1024 >= 2*512-1 evaluation points at
the L-th roots of unity:
    e   = x @ F        where F  = fft(w0, n=L, axis=1)        [512, L]
    z   = (e*e) @ Wt   where Wt = ifft(pad(wlast, L), axis=0) [L, 10]
    out = |z|
so the whole network is two dense matmuls + an elementwise complex square --
no FFT on device. F and Wt are tiny weight transforms folded on the host.

Real-expanded form on device (per core, data-parallel over batch), Gauss
3-multiplication split of the complex matmul; everything transposed (l on
partitions, batch free):
    P1 = xr@Fr ; P2 = xi@Fi ; P3 = (xr+xi)@(Fr+Fi)    (PSUM accumulation)
    m = 2*P1-P3 = er-ei ;  p = P3-2*P2 = er+ei        (DVE fused ops)
    s = p*m  = Re e^2                                 (DVE mult, bf16)
    a = p^2 ; b = m^2  (ACT squares, bf16)
    t = a-b  = 4*er*ei = 2*Im e^2                     (DVE 16-bit subtract)
    z += s@[Wtr|Wti] + t@[-Wti/2|Wtr/2]               (second matmul)
    host: out = sqrt(zr^2 + zi^2)

vs the previous revision:
  * z-stage is 4x column-tiled: the z weights are only 20 real columns, so
    four of them (s/t for an l-pair) run CONCURRENTLY in the four 32-column
    groups of the PE array via tile_position=(0, 32j), each accumulating
    into its own partition quadrant of one PSUM bank.  The four partial
    sums are added on the host.  ~14us of PE z-time drops to ~5us.
  * head restructure: the critical first-tile DMAs (F chunk0 + x batch0)
    are issued strictly in first-use order across both HWDGE queues, the
    bulk-x software-DGE streams are all dep-gated behind mid-stream
    compute (an ungated 512KB stream was stealing ~1.3us of critical-head
    HBM bandwidth), and the PE warm-up is trimmed so the PE queue isn't
    stuffed with ~5us of dummy matmuls ahead of the first real ones.
  * tail: the final batch's z packs flush as soon as their inputs exist
    and the last PSUM->SBUF copy + output DMA is split in column halves
    so the copy of one half overlaps the DMA of the other.

Sharding: pure data parallel -- batch split 8 ways, weights replicated.
"""

import numpy as np
import ml_dtypes

import concourse.bass as bass
import concourse.mybir as mybir
from concourse import bacc, tile
from concourse.bass_utils import run_bass_kernel_spmd

NCORES = 8
B, D, L, C = 16384, 512, 1024, 10
BC = B // NCORES
P = 128
BN = 512
ND = D // P
NL = L // P
NB = BC // BN
WZ = 32                  # padded z-weight columns per l-chunk (col-tiling)

F32 = mybir.dt.float32
BF16 = mybir.dt.bfloat16
ALU = mybir.AluOpType

_NC_CACHE = None


def build_nc():
    global _NC_CACHE
    if _NC_CACHE is not None:
        return _NC_CACHE

    nc = bacc.Bacc(None, target_bir_lowering=False)

    xtr_d = nc.declare_dram_parameter("xT_r", [P, NB, ND * BN], BF16,
                                      isOutput=False)
    xti_d = nc.declare_dram_parameter("xT_i", [P, NB, ND * BN], BF16,
                                      isOutput=False)
    xts_d = nc.declare_dram_parameter("xT_s", [P, NB, ND * BN], BF16,
                                      isOutput=False)
    f1_d = nc.declare_dram_parameter("F_1", [P, ND * L], BF16, isOutput=False)
    f2_d = nc.declare_dram_parameter("F_2", [P, ND * L], BF16, isOutput=False)
    f3_d = nc.declare_dram_parameter("F_3", [P, ND * L], BF16, isOutput=False)
    # z-weights: [l-part 128, NL*32]: per l-chunk 20 real columns zero-padded
    # to 32 so four of them tile the PE array's four column groups.
    w1_d = nc.declare_dram_parameter("W_1", [P, NL * WZ], BF16, isOutput=False)
    w2_d = nc.declare_dram_parameter("W_2", [P, NL * WZ], BF16, isOutput=False)
    # out: 4 partition quadrants x [zr(10)|zi(10)|pad(12)] x batch; host sums
    # the quadrants.
    out_d = nc.declare_dram_parameter("out", [P, BC], F32, isOutput=True)

    with tile.TileContext(nc) as tc:
        with (
            tc.tile_pool(name="wts", bufs=1) as wts,
            tc.tile_pool(name="xs", bufs=1) as xs,
            tc.tile_pool(name="tmp", bufs=3) as tmp,
            tc.tile_pool(name="sqf", bufs=3) as sqf,
            tc.tile_pool(name="sq", bufs=5) as sq,
            tc.tile_pool(name="zo", bufs=2) as zo,
            tc.tile_pool(name="pse", bufs=2, space="PSUM") as pse,
            tc.tile_pool(name="psz", bufs=2, space="PSUM") as psz,
        ):
            # PE warm-up (releases the HAM clock gate).  Kept short: the
            # first real matmuls land ~2us after these start and warm the
            # gate themselves; a long dummy train would sit AHEAD of them
            # in the PE queue and delay real work.
            dummy = wts.tile([P, 64], BF16, tag="dummy")
            nc.gpsimd.memset(dummy[:], 0.0)
            wacc = pse.tile([64, 64], F32, tag="p1")
            for i in range(12):
                nc.tensor.matmul(wacc[:], dummy[:, 0:64], dummy[:],
                                 start=(i == 0), stop=False,
                                 skip_group_check=True)

            def warm_fill(n):
                for _ in range(n):
                    nc.tensor.matmul(wacc[:], dummy[:, 0:64], dummy[:],
                                     start=False, stop=False,
                                     skip_group_check=True)

            f1 = wts.tile([P, ND * L], BF16, tag="f1")
            f2 = wts.tile([P, ND * L], BF16, tag="f2")
            f3 = wts.tile([P, ND * L], BF16, tag="f3")
            xtr = xs.tile([P, NB, ND * BN], BF16, tag="xtr")
            xti = xs.tile([P, NB, ND * BN], BF16, tag="xti")
            xts = xs.tile([P, NB, ND * BN], BF16, tag="xts")
            w1 = wts.tile([P, NL * WZ], BF16, tag="w1")
            w2 = wts.tile([P, NL * WZ], BF16, tag="w2")

            def flc(l):         # one l-chunk of F (l-major): 128KB
                return slice(l * D, (l + 1) * D)

            def ftail(a, b):    # l-chunks [a, b) as one fat DMA
                return slice(a * D, b * D)

            # Critical-path-first DMA order on the two HWDGE queues
            # (sync + scalar), strictly by first PE use:
            #   P1(l0) needs f1c0 + xtr b0;  P2(l0): f2c0 + xti b0;
            #   P3(l0): f3c0 + xts b0;  tiles l1.. follow F chunk arrival;
            #   first z pack needs w1/w2 (after tile l1's DVE).
            # Everything else (F tails, x b1..b3) streams behind dep gates.
            nc.sync.dma_start(f1[:, flc(0)], f1_d[:, flc(0)])
            nc.scalar.dma_start(f2[:, flc(0)], f2_d[:, flc(0)])
            nc.sync.dma_start(xtr[:, 0, 0:2 * BN], xtr_d[:, 0, 0:2 * BN])
            nc.scalar.dma_start(xti[:, 0, 0:2 * BN], xti_d[:, 0, 0:2 * BN])
            nc.sync.dma_start(xtr[:, 0, 2 * BN:], xtr_d[:, 0, 2 * BN:])
            nc.scalar.dma_start(xti[:, 0, 2 * BN:], xti_d[:, 0, 2 * BN:])
            nc.sync.dma_start(f3[:, flc(0)], f3_d[:, flc(0)])
            nc.scalar.dma_start(xts[:, 0, 0:2 * BN], xts_d[:, 0, 0:2 * BN])
            nc.sync.dma_start(xts[:, 0, 2 * BN:], xts_d[:, 0, 2 * BN:])
            nc.scalar.dma_start(f2[:, flc(1)], f2_d[:, flc(1)])
            nc.sync.dma_start(f1[:, flc(1)], f1_d[:, flc(1)])
            nc.scalar.dma_start(f3[:, flc(1)], f3_d[:, flc(1)])
            nc.sync.dma_start(w1[:], w1_d[:])
            nc.scalar.dma_start(w2[:], w2_d[:])
            nc.sync.dma_start(f1[:, ftail(2, 8)], f1_d[:, ftail(2, 8)])
            nc.scalar.dma_start(f2[:, ftail(2, 8)], f2_d[:, ftail(2, 8)])
            nc.sync.dma_start(f3[:, ftail(2, 5)], f3_d[:, ftail(2, 5)])
            nc.scalar.dma_start(f3[:, ftail(5, 8)], f3_d[:, ftail(5, 8)])

            def late_x(bstreams, dep):
                # Delay the software-DGE dispatch of bulk x until `dep` (a
                # mid-stream compute tile) exists.  A bare dma_start has no
                # dependencies and gets scheduled immediately, stealing HBM
                # bandwidth from the critical F/b0 loads in the first ~20us
                # -- so write a dep-gated byte into each destination slice
                # first; the WAW ordering paces the DMA.
                for xt, xd, b in bstreams:
                    nc.gpsimd.tensor_copy(xt[:, b, 0:1], dep)
                    nc.gpsimd.dma_start(xt[:, b, :], xd[:, b, :])

            def fsl(d, l):      # F weight chunk (d, l) in l-major packing
                return slice(l * D + d * P, l * D + (d + 1) * P)

            def wsl(l):         # z-weight slice for l-chunk (32 cols)
                return slice(l * WZ, (l + 1) * WZ)

            def dsl(d):
                return slice(d * BN, (d + 1) * BN)

            # z-stage: per batch, 4 packs; pack i contracts the l-pair
            # (2i, 2i+1): [s_2i@W1, t_2i@W2, s_2i+1@W1, t_2i+1@W2] run
            # concurrently in the 4 column groups of the PE array,
            # accumulating into partition quadrants 0..3 of one PSUM bank.
            # Host adds the quadrants.
            packs = []     # queued (zz, b, i, s0, t0, s1, t1, bs)

            def zpack(zz, b, i, st, bs):
                s0, t0, s1, t1 = st
                for j, (wt, rhs, l) in enumerate((
                        (w1, s0, 2 * i), (w2, t0, 2 * i),
                        (w1, s1, 2 * i + 1), (w2, t1, 2 * i + 1))):
                    nc.tensor.matmul(
                        zz[32 * j:32 * j + 32, :], wt[:, wsl(l)], rhs[:],
                        start=(i == 0), stop=(i == NL // 2 - 1),
                        tile_position=(0, 32 * j),
                        skip_group_check=True)
                if i == NL // 2 - 1:
                    # copy+DMA in column halves so they overlap
                    zt = zo.tile([P, BN], F32, tag="zt")
                    half = BN // 2
                    nc.scalar.copy(zt[:, 0:half], zz[:, 0:half])
                    nc.sync.dma_start(
                        out_d[:, bs.start:bs.start + half], zt[:, 0:half])
                    nc.scalar.copy(zt[:, half:], zz[:, half:])
                    nc.scalar.dma_start(
                        out_d[:, bs.start + half:bs.stop], zt[:, half:])

            warm_fill(6)
            for b in range(NB):
                bs = slice(b * BN, (b + 1) * BN)
                zz = psz.tile([P, BN], F32, tag="zz")
                stq = []
                for l in range(NL):
                    if b == 0 and l < 3:
                        warm_fill(6)
                    p1 = pse.tile([P, BN], F32, tag="p1")
                    p2 = pse.tile([P, BN], F32, tag="p2")
                    p3 = pse.tile([P, BN], F32, tag="p3")
                    for d in range(ND):
                        nc.tensor.matmul(
                            p1[:], f1[:, fsl(d, l)], xtr[:, b, dsl(d)],
                            start=(d == 0), stop=(d == ND - 1),
                            skip_group_check=True)
                    for d in range(ND):
                        nc.tensor.matmul(
                            p2[:], f2[:, fsl(d, l)], xti[:, b, dsl(d)],
                            start=(d == 0), stop=(d == ND - 1),
                            skip_group_check=True)
                    for d in range(ND):
                        nc.tensor.matmul(
                            p3[:], f3[:, fsl(d, l)], xts[:, b, dsl(d)],
                            start=(d == 0), stop=(d == ND - 1),
                            skip_group_check=True)

                    # Flush one queued z pack per tile once a safety margin
                    # of tiles separates it from the DVE work it consumes
                    # (so the PE never stalls on the DVE).  On the final
                    # batch flush as tightly as possible to shrink the tail.
                    margin = 0 if b == NB - 1 else 1
                    if len(packs) > margin:
                        zpack(*packs.pop(0))

                    # c3 = P3 (ACT copy to SBUF -- DVE stt can't take two
                    # PSUM operands) ; m = 2*P1 - c3 ; p = c3 - 2*P2  (DVE)
                    c3 = tmp.tile([P, BN], F32, tag="c3")
                    nc.scalar.copy(c3[:], p3[:])
                    m = tmp.tile([P, BN], F32, tag="m")
                    nc.vector.scalar_tensor_tensor(
                        m[:], p1[:], 2.0, c3[:], ALU.mult, ALU.subtract)
                    p = tmp.tile([P, BN], F32, tag="p")
                    nc.vector.scalar_tensor_tensor(
                        p[:], p2[:], -2.0, c3[:], ALU.mult, ALU.add)
                    # s = p*m = Re e^2 (DVE); a = p^2, bq = m^2 (ACT, bf16);
                    # t = a - bq = 2*Im e^2 (DVE, 16-bit 2x mode)
                    s = sq.tile([P, BN], BF16, tag="s")
                    nc.vector.tensor_mul(s[:], p[:], m[:])
                    a = sqf.tile([P, BN], BF16, tag="a")
                    nc.scalar.square(a[:], p[:])
                    bq = sqf.tile([P, BN], BF16, tag="bq")
                    nc.scalar.square(bq[:], m[:])
                    t = sq.tile([P, BN], BF16, tag="t")
                    nc.vector.tensor_sub(t[:], a[:], bq[:])

                    if b == 0 and l == 1:
                        late_x([(xtr, xtr_d, 1), (xti, xti_d, 1),
                                (xts, xts_d, 1)], s[:, 0:1])
                    elif b == 0 and l == 4:
                        late_x([(xtr, xtr_d, 2), (xti, xti_d, 2),
                                (xts, xts_d, 2)], s[:, 0:1])
                    elif b == 1 and l == 4:
                        late_x([(xtr, xtr_d, 3), (xti, xti_d, 3),
                                (xts, xts_d, 3)], s[:, 0:1])

                    stq.extend((s, t))
                    if l % 2 == 1:
                        packs.append((zz, b, l // 2, tuple(stq), bs))
                        stq = []

            while packs:
                zpack(*packs.pop(0))

    nc.compile()
    _NC_CACHE = nc
    return nc


def _packW(a):
    """[1024, 20] -> [128, NL*32]: per l-chunk, rows l*128..(l+1)*128 land on
    partitions, the 20 cols zero-pad to 32; chunks stack along free dim."""
    padded = np.concatenate(
        [a, np.zeros((a.shape[0], WZ - a.shape[1]))], axis=1)
    return np.ascontiguousarray(
        padded.reshape(NL, P, WZ).transpose(1, 0, 2).reshape(P, -1))


def _packF(a):
    """[512, 1024] -> [128, 4096] l-major: col l*512 + d*128 + c holds
    F[d*128+p, l*128+c], so one l-chunk's 4 contraction slices are
    contiguous and can be DMA'd just ahead of their first use."""
    return np.ascontiguousarray(
        a.reshape(ND, P, NL, P).transpose(1, 2, 0, 3).reshape(P, -1))


def _host_weights(w0_real, w0_imag, wlast_real, wlast_imag):
    w0 = w0_real.astype(np.float64) + 1j * w0_imag.astype(np.float64)
    wl = wlast_real.astype(np.float64) + 1j * wlast_imag.astype(np.float64)
    F = np.fft.fft(w0, n=L, axis=1)
    Wt = np.fft.ifft(
        np.concatenate([wl, np.zeros((1, C))], axis=0), axis=0)
    bf = ml_dtypes.bfloat16
    F1 = _packF(F.real.astype(bf))
    F2 = _packF(F.imag.astype(bf))
    F3 = _packF((F.real + F.imag).astype(bf))
    Wtr, Wti = Wt.real, Wt.imag
    W1 = _packW(np.hstack([Wtr, Wti])).astype(bf)
    W2 = _packW(np.hstack([-Wti, Wtr]) / 2.0).astype(bf)
    return F1, F2, F3, W1, W2


def make_in_maps(x_real, x_imag, w0_real, w0_imag, wlast_real, wlast_imag):
    F1, F2, F3, W1, W2 = _host_weights(
        w0_real, w0_imag, wlast_real, wlast_imag)
    bf = ml_dtypes.bfloat16
    xr = np.ascontiguousarray(x_real.T, dtype=bf)
    xi = np.ascontiguousarray(x_imag.T, dtype=bf)

    xsum = np.ascontiguousarray(
        (x_real.astype(np.float32) + x_imag.astype(np.float32)).T, dtype=bf)

    def pack3d(a):      # [512, BC] -> [128, NB, ND*BN], contiguous per b
        return np.ascontiguousarray(
            a.reshape(ND, P, NB, BN).transpose(1, 2, 0, 3).reshape(
                P, NB, ND * BN))

    in_maps = []
    for c in range(NCORES):
        sl = slice(c * BC, (c + 1) * BC)
        in_maps.append({
            "xT_r": pack3d(xr[:, sl]),
            "xT_i": pack3d(xi[:, sl]),
            "xT_s": pack3d(xsum[:, sl]),
            "F_1": F1, "F_2": F2, "F_3": F3,
            "W_1": W1, "W_2": W2,
        })
    return in_maps


def postprocess(results):
    outs = []
    for c in range(NCORES):
        o = results[c]["out"]
        # sum the 4 PE column-group quadrants, then |z|
        z = (o[0:2 * C] + o[32:32 + 2 * C]
             + o[64:64 + 2 * C] + o[96:96 + 2 * C])
        mag = np.sqrt(z[:C] ** 2 + z[C:2 * C] ** 2).T
        outs.append(mag)
    return np.ascontiguousarray(np.concatenate(outs, axis=0), dtype=np.float32)


def kernel(x_real, x_imag, w0_real, w0_imag, wlast_real, wlast_imag):
    x_real, x_imag, w0_real, w0_imag, wlast_real, wlast_imag = (
        np.asarray(arr) for arr in
        (x_real, x_imag, w0_real, w0_imag, wlast_real, wlast_imag))
    nc = build_nc()
    in_maps = make_in_maps(
        x_real, x_imag, w0_real, w0_imag, wlast_real, wlast_imag)
    # A stale/wedged NeuronCore (e.g. a previously killed process that died
    # mid-execute) fails with NRT_EXEC_UNIT_UNRECOVERABLE; reloading resets
    # it but may need a fresh backend and a moment. Retry a few times.
    import time
    last = None
    for attempt in range(4):
        try:
            res = run_bass_kernel_spmd(
                nc, in_maps, core_ids=list(range(NCORES)))
            return postprocess(res.results)
        except Exception as e:
            last = e
            time.sleep(2.0 + 2.0 * attempt)
            try:
                import jax
                import jax.extend.backend
                jax.clear_caches()
                jax.extend.backend.clear_backends()
            except Exception:
                pass
    raise last


# revision 3
# speedup vs baseline: 1.0613x; 1.0613x over previous
"""Trainium2 Bass kernel for nn_ANN_Comp_29240137351521 (dense_cnn).

Reference computes, per batch row b of x [16384, 512] (complex, given as
real/imag f32 pairs):
    h = x @ w0                      # [B, 512] complex
    a = ifft(fft(h, n=1023)^2)      # full self-convolution, [B, 1023]
    out = |a @ wlast|               # [B, 10] f32

Algebraic collapse used here: the self-convolution + final contraction is a
polynomial-evaluation identity. With L = 1024 >= 2*512-1 evaluation points at
the L-th roots of unity:
    e   = x @ F        where F  = fft(w0, n=L, axis=1)        [512, L]
    z   = (e*e) @ Wt   where Wt = ifft(pad(wlast, L), axis=0) [L, 10]
    out = |z|
so the whole network is two dense matmuls + an elementwise complex square --
no FFT on device. F and Wt are tiny weight transforms folded on the host.

Real-expanded form on device (per core, data-parallel over batch), Gauss
3-multiplication split of the complex matmul; everything transposed (l on
partitions, batch free):
    P1 = xr@Fr ; P2 = xi@Fi ; P3 = (xr+xi)@(Fr+Fi)    (PSUM accumulation)
    m = 2*P1-P3 = er-ei ;  p = P3-2*P2 = er+ei        (DVE fused ops)
    s = p*m  = Re e^2                                 (DVE mult, bf16)
    a = p^2 ; b = m^2  (ACT squares, bf16)
    t = a-b  = 4*er*ei = 2*Im e^2                     (DVE 16-bit subtract)
    z += s@[Wtr|Wti] + t@[-Wti/2|Wtr/2]               (second matmul)
    host: out = sqrt(zr^2 + zi^2)

Scheduling notes (the measured-trace rationale):
  * z-stage is 4x column-tiled: the z weights are only 20 real columns, so
    four z matmuls (s/t for an l-pair) run CONCURRENTLY in the four
    32-column groups of the PE array via tile_position=(0, 32j), each
    accumulating into its own partition quadrant of one PSUM bank (pack
    span measured 400ns vs 864ns serial).  Host adds the quadrants.
  * All loads ride the two HWDGE queues in strict first-use order; each
    queue is FIFO, so later bulk entries can't steal bandwidth from the
    critical head (no software-DGE, no dep-gate tricks needed).  F tails
    are per-l-chunk DMAs -- a fat multi-chunk DMA only fires its
    completion semaphore at the END, which stalled tiles l>=2 ~7us.
  * The scalar (ACT) engine dispatches only the 8 head-critical DMAs; it
    must be free for PSUM->SBUF copies by ~12us or the p3 PSUM ring
    recycling stalls the PE.  Everything else dispatches from sync.
  * x_sum for batch 0 is computed on the (then-idle) DVE instead of being
    DMA'd: the head critical set shrinks 512KB, pulling the first full
    tile ~1.3us earlier.  Batches 1-3 stream xts from HBM as before.
  * The final tile's DVE chain, z pack, PSUM copy and output DMA are all
    split into column halves so each stage of the second half overlaps
    the next stage of the first.

Sharding: pure data parallel -- batch split 8 ways, weights replicated.
"""

import numpy as np
import ml_dtypes

import concourse.bass as bass
import concourse.mybir as mybir
from concourse import bacc, tile
from concourse.bass_utils import run_bass_kernel_spmd

NCORES = 8
B, D, L, C = 16384, 512, 1024, 10
BC = B // NCORES
P = 128
BN = 512
ND = D // P
NL = L // P
NB = BC // BN
WZ = 32                  # padded z-weight columns per l-chunk (col-tiling)

F32 = mybir.dt.float32
BF16 = mybir.dt.bfloat16
ALU = mybir.AluOpType

_NC_CACHE = None


def build_nc():
    global _NC_CACHE
    if _NC_CACHE is not None:
        return _NC_CACHE

    nc = bacc.Bacc(None, target_bir_lowering=False)

    xtr_d = nc.declare_dram_parameter("xT_r", [P, NB, ND * BN], BF16,
                                      isOutput=False)
    xti_d = nc.declare_dram_parameter("xT_i", [P, NB, ND * BN], BF16,
                                      isOutput=False)
    xts_d = nc.declare_dram_parameter("xT_s", [P, NB, ND * BN], BF16,
                                      isOutput=False)
    f1_d = nc.declare_dram_parameter("F_1", [P, ND * L], BF16, isOutput=False)
    f2_d = nc.declare_dram_parameter("F_2", [P, ND * L], BF16, isOutput=False)
    f3_d = nc.declare_dram_parameter("F_3", [P, ND * L], BF16, isOutput=False)
    # z-weights: [l-part 128, NL*32]: per l-chunk 20 real columns zero-padded
    # to 32 so four of them tile the PE array's four column groups.
    w1_d = nc.declare_dram_parameter("W_1", [P, NL * WZ], BF16, isOutput=False)
    w2_d = nc.declare_dram_parameter("W_2", [P, NL * WZ], BF16, isOutput=False)
    # out: 4 partition quadrants x [zr(10)|zi(10)|pad(12)] x batch; host sums
    # the quadrants.
    out_d = nc.declare_dram_parameter("out", [P, BC], F32, isOutput=True)

    with tile.TileContext(nc) as tc:
        with (
            tc.tile_pool(name="wts", bufs=1) as wts,
            tc.tile_pool(name="xs", bufs=1) as xs,
            tc.tile_pool(name="tmp", bufs=3) as tmp,
            tc.tile_pool(name="sqf", bufs=3) as sqf,
            tc.tile_pool(name="sq", bufs=5) as sq,
            tc.tile_pool(name="zo", bufs=2) as zo,
            tc.tile_pool(name="pse", bufs=2, space="PSUM") as pse,
            tc.tile_pool(name="psz", bufs=2, space="PSUM") as psz,
        ):
            # PE warm-up (releases the HAM clock gate).  Kept short: the
            # first real matmuls land ~2us after these start and warm the
            # gate themselves; a long dummy train would sit AHEAD of them
            # in the PE queue and delay real work.
            dummy = wts.tile([P, 64], BF16, tag="dummy")
            nc.gpsimd.memset(dummy[:], 0.0)
            wacc = pse.tile([64, 64], F32, tag="p1")
            for i in range(12):
                nc.tensor.matmul(wacc[:], dummy[:, 0:64], dummy[:],
                                 start=(i == 0), stop=False,
                                 skip_group_check=True)

            def warm_fill(n):
                for _ in range(n):
                    nc.tensor.matmul(wacc[:], dummy[:, 0:64], dummy[:],
                                     start=False, stop=False,
                                     skip_group_check=True)

            f1 = wts.tile([P, ND * L], BF16, tag="f1")
            f2 = wts.tile([P, ND * L], BF16, tag="f2")
            f3 = wts.tile([P, ND * L], BF16, tag="f3")
            xtr = xs.tile([P, NB, ND * BN], BF16, tag="xtr")
            xti = xs.tile([P, NB, ND * BN], BF16, tag="xti")
            xts = xs.tile([P, NB, ND * BN], BF16, tag="xts")
            w1 = wts.tile([P, NL * WZ], BF16, tag="w1")
            w2 = wts.tile([P, NL * WZ], BF16, tag="w2")

            def flc(l):         # one l-chunk of F (l-major): 128KB
                return slice(l * D, (l + 1) * D)

            # --- head: critical first-tile loads, first-use order.
            # sync queue: P1's needs, then f3c0/f1c1/w1.
            nc.sync.dma_start(f1[:, flc(0)], f1_d[:, flc(0)])
            nc.scalar.dma_start(f2[:, flc(0)], f2_d[:, flc(0)])
            nc.sync.dma_start(xtr[:, 0, 0:2 * BN], xtr_d[:, 0, 0:2 * BN])
            nc.scalar.dma_start(xti[:, 0, 0:2 * BN], xti_d[:, 0, 0:2 * BN])
            nc.sync.dma_start(xtr[:, 0, 2 * BN:], xtr_d[:, 0, 2 * BN:])
            nc.scalar.dma_start(xti[:, 0, 2 * BN:], xti_d[:, 0, 2 * BN:])
            nc.sync.dma_start(f3[:, flc(0)], f3_d[:, flc(0)])
            nc.scalar.dma_start(f2[:, flc(1)], f2_d[:, flc(1)])
            nc.sync.dma_start(f1[:, flc(1)], f1_d[:, flc(1)])
            nc.scalar.dma_start(f3[:, flc(1)], f3_d[:, flc(1)])
            nc.sync.dma_start(w1[:], w1_d[:])
            nc.scalar.dma_start(w2[:], w2_d[:])
            # --- F tails: one DMA per l-chunk (per-chunk completion
            # semaphores), interleaved by first use, all on sync so the
            # scalar engine is free for ACT work when tile0 finishes.
            for l in range(2, NL):
                nc.sync.dma_start(f1[:, flc(l)], f1_d[:, flc(l)])
                nc.sync.dma_start(f2[:, flc(l)], f2_d[:, flc(l)])
                nc.sync.dma_start(f3[:, flc(l)], f3_d[:, flc(l)])
            # --- bulk x, FIFO-paced behind the above (no bandwidth
            # stealing possible within a queue).  b1 split across both
            # queues (needed ~30us), b2/b3 on sync.
            nc.scalar.dma_start(xtr[:, 1, :], xtr_d[:, 1, :])
            nc.scalar.dma_start(xti[:, 1, :], xti_d[:, 1, :])
            nc.sync.dma_start(xts[:, 1, :], xts_d[:, 1, :])
            for b in (2, 3):
                nc.sync.dma_start(xtr[:, b, :], xtr_d[:, b, :])
                nc.sync.dma_start(xti[:, b, :], xti_d[:, b, :])
                nc.sync.dma_start(xts[:, b, :], xts_d[:, b, :])

            # xts b0 on the (idle-until-~12us) DVE instead of HBM:
            # halves so each can start as soon as its xtr/xti half lands.
            nc.vector.tensor_add(xts[:, 0, 0:2 * BN],
                                 xtr[:, 0, 0:2 * BN], xti[:, 0, 0:2 * BN])
            nc.vector.tensor_add(xts[:, 0, 2 * BN:],
                                 xtr[:, 0, 2 * BN:], xti[:, 0, 2 * BN:])

            def fsl(d, l):      # F weight chunk (d, l) in l-major packing
                return slice(l * D + d * P, l * D + (d + 1) * P)

            def wsl(l):         # z-weight slice for l-chunk (32 cols)
                return slice(l * WZ, (l + 1) * WZ)

            def dsl(d):
                return slice(d * BN, (d + 1) * BN)

            # z-stage: per batch, 4 packs; pack i contracts the l-pair
            # (2i, 2i+1): [s_2i@W1, t_2i@W2, s_2i+1@W1, t_2i+1@W2] run
            # concurrently in the 4 column groups of the PE array,
            # accumulating into partition quadrants 0..3 of one PSUM bank.
            # Host adds the quadrants.  cols selects a free-dim slice for
            # the split final pack.
            packs = []     # queued (zz, b, i, (s0,t0,s1,t1), bs)

            def zmms(zz, i, st, cols, stop):
                s0, t0, s1, t1 = st
                for j, (wt, rhs, l) in enumerate((
                        (w1, s0, 2 * i), (w2, t0, 2 * i),
                        (w1, s1, 2 * i + 1), (w2, t1, 2 * i + 1))):
                    nc.tensor.matmul(
                        zz[32 * j:32 * j + 32, cols], wt[:, wsl(l)],
                        rhs[:, cols],
                        start=(i == 0), stop=stop,
                        tile_position=(0, 32 * j),
                        skip_group_check=True)

            def zpack(zz, b, i, st, bs):
                last = i == NL // 2 - 1
                zmms(zz, i, st, slice(0, BN), last)
                if last:
                    # copy+DMA in column halves so they overlap
                    zt = zo.tile([P, BN], F32, tag="zt")
                    half = BN // 2
                    nc.scalar.copy(zt[:, 0:half], zz[:, 0:half])
                    nc.sync.dma_start(
                        out_d[:, bs.start:bs.start + half], zt[:, 0:half])
                    nc.scalar.copy(zt[:, half:], zz[:, half:])
                    nc.scalar.dma_start(
                        out_d[:, bs.start + half:bs.stop], zt[:, half:])

            def dve_chain(p1, p2, p3, cols):
                """c3/m/p/s/a/bq/t for a free-dim column slice; returns s, t
                tiles (full-width tiles, the slice written)."""
                c3 = tmp.tile([P, BN], F32, tag="c3")
                nc.scalar.copy(c3[:, cols], p3[:, cols])
                m = tmp.tile([P, BN], F32, tag="m")
                nc.vector.scalar_tensor_tensor(
                    m[:, cols], p1[:, cols], 2.0, c3[:, cols],
                    ALU.mult, ALU.subtract)
                p = tmp.tile([P, BN], F32, tag="p")
                nc.vector.scalar_tensor_tensor(
                    p[:, cols], p2[:, cols], -2.0, c3[:, cols],
                    ALU.mult, ALU.add)
                s = sq.tile([P, BN], BF16, tag="s")
                nc.vector.tensor_mul(s[:, cols], p[:, cols], m[:, cols])
                a = sqf.tile([P, BN], BF16, tag="a")
                nc.scalar.square(a[:, cols], p[:, cols])
                bq = sqf.tile([P, BN], BF16, tag="bq")
                nc.scalar.square(bq[:, cols], m[:, cols])
                t = sq.tile([P, BN], BF16, tag="t")
                nc.vector.tensor_sub(t[:, cols], a[:, cols], bq[:, cols])
                return s, t

            warm_fill(6)
            for b in range(NB):
                bs = slice(b * BN, (b + 1) * BN)
                zz = psz.tile([P, BN], F32, tag="zz")
                stq = []
                for l in range(NL):
                    if b == 0 and l < 3:
                        warm_fill(6)
                    lastt = b == NB - 1 and l == NL - 1
                    p1 = pse.tile([P, BN], F32, tag="p1")
                    p2 = pse.tile([P, BN], F32, tag="p2")
                    p3 = pse.tile([P, BN], F32, tag="p3")
                    for d in range(ND):
                        nc.tensor.matmul(
                            p1[:], f1[:, fsl(d, l)], xtr[:, b, dsl(d)],
                            start=(d == 0), stop=(d == ND - 1),
                            skip_group_check=True)
                    for d in range(ND):
                        nc.tensor.matmul(
                            p2[:], f2[:, fsl(d, l)], xti[:, b, dsl(d)],
                            start=(d == 0), stop=(d == ND - 1),
                            skip_group_check=True)
                    for d in range(ND):
                        nc.tensor.matmul(
                            p3[:], f3[:, fsl(d, l)], xts[:, b, dsl(d)],
                            start=(d == 0), stop=(d == ND - 1),
                            skip_group_check=True)

                    # Flush one queued z pack per tile once a safety margin
                    # of tiles separates it from the DVE work it consumes
                    # (so the PE never stalls on the DVE).  On the final
                    # batch flush as tightly as possible to shrink the tail.
                    margin = 0 if b == NB - 1 else 1
                    if len(packs) > margin:
                        zpack(*packs.pop(0))

                    if not lastt:
                        s, t = dve_chain(p1, p2, p3, slice(0, BN))
                        stq.extend((s, t))
                        if l % 2 == 1:
                            packs.append((zz, b, l // 2, tuple(stq), bs))
                            stq = []
                    else:
                        # final tile: column-half pipeline straight through
                        # DVE chain -> half z pack -> half copy -> half DMA
                        half = BN // 2
                        zt = zo.tile([P, BN], F32, tag="zt")
                        s0, t0 = stq
                        for hi, cols in enumerate(
                                (slice(0, half), slice(half, BN))):
                            s, t = dve_chain(p1, p2, p3, cols)
                            zmms(zz, NL // 2 - 1, (s0, t0, s, t), cols, True)
                            nc.scalar.copy(zt[:, cols], zz[:, cols])
                            eng = nc.sync if hi == 0 else nc.scalar
                            eng.dma_start(
                                out_d[:, b * BN + cols.start:
                                      b * BN + cols.stop], zt[:, cols])

            while packs:
                zpack(*packs.pop(0))

    nc.compile()
    _NC_CACHE = nc
    return nc


def _packW(a):
    """[1024, 20] -> [128, NL*32]: per l-chunk, rows l*128..(l+1)*128 land on
    partitions, the 20 cols zero-pad to 32; chunks stack along free dim."""
    padded = np.concatenate(
        [a, np.zeros((a.shape[0], WZ - a.shape[1]))], axis=1)
    return np.ascontiguousarray(
        padded.reshape(NL, P, WZ).transpose(1, 0, 2).reshape(P, -1))


def _packF(a):
    """[512, 1024] -> [128, 4096] l-major: col l*512 + d*128 + c holds
    F[d*128+p, l*128+c], so one l-chunk's 4 contraction slices are
    contiguous and can be DMA'd just ahead of their first use."""
    return np.ascontiguousarray(
        a.reshape(ND, P, NL, P).transpose(1, 2, 0, 3).reshape(P, -1))


def _host_weights(w0_real, w0_imag, wlast_real, wlast_imag):
    w0 = w0_real.astype(np.float64) + 1j * w0_imag.astype(np.float64)
    wl = wlast_real.astype(np.float64) + 1j * wlast_imag.astype(np.float64)
    F = np.fft.fft(w0, n=L, axis=1)
    Wt = np.fft.ifft(
        np.concatenate([wl, np.zeros((1, C))], axis=0), axis=0)
    bf = ml_dtypes.bfloat16
    F1 = _packF(F.real.astype(bf))
    F2 = _packF(F.imag.astype(bf))
    F3 = _packF((F.real + F.imag).astype(bf))
    Wtr, Wti = Wt.real, Wt.imag
    W1 = _packW(np.hstack([Wtr, Wti])).astype(bf)
    W2 = _packW(np.hstack([-Wti, Wtr]) / 2.0).astype(bf)
    return F1, F2, F3, W1, W2


def make_in_maps(x_real, x_imag, w0_real, w0_imag, wlast_real, wlast_imag):
    F1, F2, F3, W1, W2 = _host_weights(
        w0_real, w0_imag, wlast_real, wlast_imag)
    bf = ml_dtypes.bfloat16
    xr = np.ascontiguousarray(x_real.T, dtype=bf)
    xi = np.ascontiguousarray(x_imag.T, dtype=bf)

    xsum = np.ascontiguousarray(
        (x_real.astype(np.float32) + x_imag.astype(np.float32)).T, dtype=bf)

    def pack3d(a):      # [512, BC] -> [128, NB, ND*BN], contiguous per b
        return np.ascontiguousarray(
            a.reshape(ND, P, NB, BN).transpose(1, 2, 0, 3).reshape(
                P, NB, ND * BN))

    in_maps = []
    for c in range(NCORES):
        sl = slice(c * BC, (c + 1) * BC)
        in_maps.append({
            "xT_r": pack3d(xr[:, sl]),
            "xT_i": pack3d(xi[:, sl]),
            "xT_s": pack3d(xsum[:, sl]),
            "F_1": F1, "F_2": F2, "F_3": F3,
            "W_1": W1, "W_2": W2,
        })
    return in_maps


def postprocess(results):
    outs = []
    for c in range(NCORES):
        o = results[c]["out"]
        # sum the 4 PE column-group quadrants, then |z|
        z = (o[0:2 * C] + o[32:32 + 2 * C]
             + o[64:64 + 2 * C] + o[96:96 + 2 * C])
        mag = np.sqrt(z[:C] ** 2 + z[C:2 * C] ** 2).T
        outs.append(mag)
    return np.ascontiguousarray(np.concatenate(outs, axis=0), dtype=np.float32)


def kernel(x_real, x_imag, w0_real, w0_imag, wlast_real, wlast_imag):
    x_real, x_imag, w0_real, w0_imag, wlast_real, wlast_imag = (
        np.asarray(arr) for arr in
        (x_real, x_imag, w0_real, w0_imag, wlast_real, wlast_imag))
    nc = build_nc()
    in_maps = make_in_maps(
        x_real, x_imag, w0_real, w0_imag, wlast_real, wlast_imag)
    # A stale/wedged NeuronCore (e.g. a previously killed process that died
    # mid-execute) fails with NRT_EXEC_UNIT_UNRECOVERABLE; reloading resets
    # it but may need a fresh backend and a moment. Retry a few times.
    import time
    last = None
    for attempt in range(4):
        try:
            res = run_bass_kernel_spmd(
                nc, in_maps, core_ids=list(range(NCORES)))
            return postprocess(res.results)
        except Exception as e:
            last = e
            time.sleep(2.0 + 2.0 * attempt)
            try:
                import jax
                import jax.extend.backend
                jax.clear_caches()
                jax.extend.backend.clear_backends()
            except Exception:
                pass
    raise last
